# revision 36
# baseline (speedup 1.0000x reference)
"""Trainium2 Bass kernel for AdaptiveGraphLearning (retrieval_knn).

For X [8192,128], A_raw [8192,8192], lambda scalar:
  Xn = X / max(||X||_2, 1e-12);  S = Xn @ Xn.T
  A  = dense top-(K+1) per row, self-edge dropped, row-normalized
  A_final = sigmoid(lam)*A_raw + (1-sigmoid(lam))*A_learned

Distribution: row-shard N across 8 cores (1024 rows each). The host
pre-normalizes X and ships Xn^T (replicated, [128, 8192]) plus each
core's own row-block slice; the device computes its [1024, 8192]
similarity block with fp32r matmuls (1 cycle/row -- 4x the fp32 rate),
finds each row's rank-11 threshold tau via per-chunk max8 candidates,
and streams out zsel = relu(S - tau'') in bf16, where tau'' = tau*(1 -
2^-9). The downshifted threshold makes every column within ~5e-4 of the
boundary visible in zsel, so the host can repair fp32r's ~1e-5 rounding
exactly: columns inside a +-4e-4 band around tau are recomputed with an
exact dot product and re-ranked so the selected set matches full-fp32
top-k. Everything downstream of the select (row-normalize, the affine
combine with A_raw, diagonal removal) is dense streaming work the host
applies while gathering.

Device engine split per row-tile: PE does 16 fp32r matmuls into
[P,2048] PSUM tiles; ACT drains three of four to SBUF as bf16 and DVE
the fourth; DVE max8-scans only the first 512 columns of each PSUM
chunk (a 1/4 sample -- tau~ lands near true rank ~44, which only
widens the host repair band, never misses a member), runs the tiny
top-16 tournament for tau in f32, and computes all four select
quarters with the 2-op tensor_scalar on bf16 at the 4x packed rate.
"""

import numpy as np

N = 8192
D = 128
NCORES = 8
RPC = N // NCORES   # rows per core
P = 128
TILES = RPC // P    # row tiles per core
MMF = 512           # matmul moving free dim (one PSUM bank, f32)
CH = 1024           # PSUM chunk width (two banks)
NCH = N // CH       # chunks per row: 8
CAND = 16           # candidates per row (top-8 of chunks 0 and 1)
ZQ = 2048           # zsel quarter width
NZQ = N // ZQ
SCW = 512           # scanned prefix of each PSUM chunk (1/4 sample)
SHIFT = np.float32(1.0 - 2.0 ** -7)   # tau'' = tau * SHIFT
BAND = np.float32(0.09)               # host exact-recompute band above tau
K1 = 11                               # top-(k+1) incl self

LAST_RESULTS = None
_NC_CACHE = None


def _build():
    import concourse.mybir as mybir
    import concourse.tile as tile
    from concourse import bacc
    from concourse.bass import ts

    f32 = mybir.dt.float32
    f32r = mybir.dt.float32r
    bf16 = mybir.dt.bfloat16
    AF = mybir.ActivationFunctionType
    OP = mybir.AluOpType

    nc = bacc.Bacc("TRN2", target_bir_lowering=False, debug=False,
                   num_devices=NCORES)

    XNT_d = nc.dram_tensor("xnt", [P, N], bf16, kind="ExternalInput")
    XR_d = nc.dram_tensor("xrows", [P, RPC], bf16, kind="ExternalInput")
    ZS_d = nc.dram_tensor("zsel", [RPC, N], bf16, kind="ExternalOutput")
    TAU_d = nc.dram_tensor("tau", [P, TILES], f32, kind="ExternalOutput")
    TAU2_d = nc.dram_tensor("tau2", [P, TILES], f32, kind="ExternalOutput")

    with tile.TileContext(nc) as tc:
        with (
            tc.tile_pool(name="xp", bufs=1) as xp,
            tc.tile_pool(name="sp", bufs=2) as sp,
            tc.tile_pool(name="zp", bufs=2) as zp,
            tc.tile_pool(name="small", bufs=2) as smallp,
            tc.tile_pool(name="const", bufs=1) as constp,
            tc.tile_pool(name="psum", bufs=4, space="PSUM") as psump,
        ):
            # xrows first: every matmul's stationary operand needs it
            xrows = xp.tile([P, RPC], bf16, name="xrows")
            nc.sync.dma_start(xrows[:], XR_d.ap())
            xnt = xp.tile([P, N], bf16, name="xnt")
            for g in range(8):
                ring = nc.sync if g % 2 == 0 else nc.scalar
                ring.dma_start(xnt[:, ts(g, N // 8)],
                               XNT_d.ap()[:, ts(g, N // 8)])

            taus = constp.tile([P, TILES], f32, name="taus")
            tau2 = constp.tile([P, TILES], f32, name="tau2")
            ntau2 = constp.tile([P, TILES], f32, name="ntau2")

            for t in range(TILES):
                cand = smallp.tile([P, CAND], f32, name=f"cand{t}",
                                   tag="cand")
                z_t = zp.tile([P, N], bf16, name=f"z{t}", tag="z")
                pm01 = []
                for c in range(NCH):
                    pm = psump.tile([P, CH], f32, name=f"pm{t}_{c}",
                                    tag="mm")
                    for k in range(CH // MMF):
                        nc.tensor.matmul(pm[:, ts(k, MMF)],
                                         xrows[:, ts(t, P)],
                                         xnt[:, ts(c * (CH // MMF) + k, MMF)],
                                         start=True, stop=True)
                    if c < 2:
                        # sampled scan straight from PSUM (f32): chunks 0+1
                        # = a fixed 1/4 column sample, so tau~ is ready two
                        # chunks into the window; the chunk stays in PSUM
                        # until its select right after the tournament
                        nc.vector.max(cand[:, ts(c, 8)], pm[:])
                        pm01.append(pm)
                    elif c < 6:
                        # drain+select fused on ACT: relu(S - tau') from
                        # PSUM, bf16 out
                        nc.scalar.activation(z_t[:, ts(c, CH)], pm[:],
                                             AF.Relu,
                                             bias=ntau2[:, t:t + 1],
                                             scale=1.0)
                    else:
                        # same fusion on DVE for the last two chunks
                        nc.vector.tensor_scalar(z_t[:, ts(c, CH)], pm[:],
                                                tau2[:, t:t + 1], 0.0,
                                                OP.subtract, OP.max)
                    if c == 1:
                        # tau tournament right after the two sampled scans
                        g12 = smallp.tile([P, 16], f32, name=f"g12_{t}",
                                          tag="g12")
                        nc.vector.max(g12[:, 0:8], cand[:])
                        nc.vector.match_replace(out=cand[:],
                                                in_to_replace=g12[:, 0:8],
                                                in_values=cand[:],
                                                imm_value=-1e30)
                        nc.vector.max(g12[:, 8:16], cand[:])
                        nc.vector.tensor_copy(taus[:, t:t + 1],
                                              g12[:, 10:11])
                        nc.vector.tensor_scalar_mul(tau2[:, t:t + 1],
                                                    taus[:, t:t + 1],
                                                    float(SHIFT))
                        nc.vector.tensor_scalar_mul(ntau2[:, t:t + 1],
                                                    taus[:, t:t + 1],
                                                    -float(SHIFT))
                        # chunks 0-1 select from their still-live PSUM
                        for cc, pmx in enumerate(pm01):
                            nc.vector.tensor_scalar(z_t[:, ts(cc, CH)],
                                                    pmx[:],
                                                    tau2[:, t:t + 1], 0.0,
                                                    OP.subtract, OP.max)
                    if c == 3:
                        nc.sync.dma_start(ZS_d.ap()[ts(t, P), 0:4 * CH],
                                          z_t[:, 0:4 * CH])
                    elif c == 7:
                        nc.sync.dma_start(ZS_d.ap()[ts(t, P), 4 * CH:N],
                                          z_t[:, 4 * CH:N])

            nc.sync.dma_start(TAU_d.ap(), taus[:])
            nc.sync.dma_start(TAU2_d.ap(), tau2[:])

    nc.compile()
    return nc


def kernel(X, A_raw, lambda_param):
    global LAST_RESULTS, _NC_CACHE
    import ml_dtypes
    from concourse.bass_utils import run_bass_kernel_spmd

    X = np.asarray(X, dtype=np.float32)
    A_raw = np.asarray(A_raw, dtype=np.float32)
    lam = float(np.asarray(lambda_param, dtype=np.float32).reshape(()))

    if _NC_CACHE is None:
        _NC_CACHE = _build()
    nc = _NC_CACHE

    norms = np.maximum(np.linalg.norm(X, axis=1, keepdims=True),
                       np.float32(1e-12)).astype(np.float32)
    Xn = (X / norms).astype(np.float32)
    XnT = np.ascontiguousarray(Xn.T)           # [128, 8192]
    XnT16 = XnT.astype(ml_dtypes.bfloat16)
    in_maps = []
    for c in range(NCORES):
        r0 = c * RPC
        in_maps.append({
            "xnt": XnT16,
            "xrows": np.ascontiguousarray(XnT16[:, r0:r0 + RPC]),
        })

    res = run_bass_kernel_spmd(nc, in_maps, core_ids=list(range(NCORES)))
    LAST_RESULTS = res

    zs = np.empty((N, N), dtype=np.float32)
    tau = np.empty((N, 1), dtype=np.float32)
    tau2 = np.empty((N, 1), dtype=np.float32)
    for c in range(NCORES):
        r0 = c * RPC
        zs[r0:r0 + RPC] = np.asarray(res.results[c]["zsel"],
                                     dtype=np.float32)
        # [P, TILES] with local row t*128+p -> transpose+flatten
        tau[r0:r0 + RPC, 0] = res.results[c]["tau"].T.reshape(RPC)
        tau2[r0:r0 + RPC, 0] = res.results[c]["tau2"].T.reshape(RPC)

    pos = zs > 0                    # everything at or above tau'' (dense)
    s_up = np.where(pos, zs + tau2, np.float32(-2.0))   # approx S, else -2

    # Exact repair band: recompute every visible column within BAND of
    # tau with a full-precision dot product (fp32r noise is ~1e-5; the
    # relu shift tau-tau'' ~5e-4 guarantees all true top-11 columns are
    # visible). Typically ~0.3 columns/row land here.
    band = pos & (s_up <= tau + BAND)
    brows, bcols = np.nonzero(band)
    if brows.size:
        exact = np.empty(brows.size, dtype=np.float32)
        CKB = 1 << 20
        for o in range(0, brows.size, CKB):
            r, c = brows[o:o + CKB], bcols[o:o + CKB]
            exact[o:o + CKB] = np.einsum(
                "ij,ij->i", Xn[r].astype(np.float64),
                Xn[c].astype(np.float64)).astype(np.float32)
        s_up[brows, bcols] = exact

    clear = pos & (s_up > tau + BAND)
    cnt = clear.sum(axis=1)

    # pick (11 - clear_count) more per row from the band, by exact value
    need = K1 - cnt
    mask = clear
    if brows.size:
        bvals = s_up[brows, bcols]
        order = np.lexsort((bcols, -bvals, brows))
        br_s, bc_s = brows[order], bcols[order]
        # occurrence rank of each band entry within its row
        first = np.r_[True, br_s[1:] != br_s[:-1]]
        idx = np.arange(br_s.size)
        start = np.maximum.accumulate(np.where(first, idx, 0))
        occ = idx - start
        take = occ < need[br_s]
        mask = mask.copy()
        mask[br_s[take], bc_s[take]] = True

    # rare pathologies (scan missed a dense cluster, exact f32 ties):
    # any row whose selected count != 11 gets a full exact re-rank
    bad = np.nonzero(mask.sum(axis=1) != K1)[0]
    for r in bad:
        cols = np.nonzero(pos[r])[0]
        ex = (Xn[cols].astype(np.float64) @ Xn[r].astype(np.float64))
        top = cols[np.argsort(-ex, kind="stable")[:K1]]
        mask[r, :] = False
        mask[r, top] = True
        s_up[r, top] = ex[np.argsort(-ex, kind="stable")[:K1]].astype(
            np.float32)

    idx = np.arange(N)
    mask[idx, idx] = False          # drop the self-edge (10 left per row)

    sel = np.where(mask, s_up, np.float32(0.0))
    den = sel.sum(axis=1, keepdims=True) + np.float32(1e-6)
    A_learned = sel / den
    sig = np.float32(1.0 / (1.0 + np.exp(-lam)))
    A_final = sig * A_raw + (np.float32(1.0) - sig) * A_learned
    return A_final, A_learned


# revision 45
# speedup vs baseline: 1.2080x; 1.2080x over previous
"""Trainium2 Bass kernel for AdaptiveGraphLearning (retrieval_knn).

For X [8192,128], A_raw [8192,8192], lambda scalar:
  Xn = X / max(||X||_2, 1e-12);  S = Xn @ Xn.T
  A  = dense top-(K+1) per row, self-edge dropped, row-normalized
  A_final = sigmoid(lam)*A_raw + (1-sigmoid(lam))*A_learned

Distribution: row-shard N across 8 cores (1024 rows each). The host
pre-normalizes X and ships Xn^T (replicated, [128, 8192]) plus each
core's own row-block slice; the device computes its [1024, 8192]
similarity block with fp32r matmuls (1 cycle/row -- 4x the fp32 rate),
finds each row's rank-11 threshold tau via per-chunk max8 candidates,
and streams out zsel = relu(S - tau'') in bf16, where tau'' = tau*(1 -
2^-9). The downshifted threshold makes every column within ~5e-4 of the
boundary visible in zsel, so the host can repair fp32r's ~1e-5 rounding
exactly: columns inside a +-4e-4 band around tau are recomputed with an
exact dot product and re-ranked so the selected set matches full-fp32
top-k. Everything downstream of the select (row-normalize, the affine
combine with A_raw, diagonal removal) is dense streaming work the host
applies while gathering.

Device engine split per row-tile: PE does 16 fp32r matmuls into
[P,2048] PSUM tiles; ACT drains three of four to SBUF as bf16 and DVE
the fourth; DVE max8-scans only the first 512 columns of each PSUM
chunk (a 1/4 sample -- tau~ lands near true rank ~44, which only
widens the host repair band, never misses a member), runs the tiny
top-16 tournament for tau in f32, and computes all four select
quarters with the 2-op tensor_scalar on bf16 at the 4x packed rate.
"""

import numpy as np

N = 8192
D = 128
NCORES = 8
RPC = N // NCORES   # rows per core
P = 128
TILES = RPC // P    # row tiles per core
MMF = 512           # matmul moving free dim (one PSUM bank, f32)
CH = 1024           # PSUM chunk width (two banks)
NCH = N // CH       # chunks per row: 8
CAND = 16           # candidates per row (top-8 of chunks 0 and 1)
ZQ = 2048           # zsel quarter width
NZQ = N // ZQ
SCW = 512           # scanned prefix of each PSUM chunk (1/4 sample)
SHIFT = np.float32(1.0 - 2.0 ** -7)   # tau'' = tau * SHIFT
BAND = np.float32(0.09)               # host exact-recompute band above tau
K1 = 11                               # top-(k+1) incl self

LAST_RESULTS = None
_NC_CACHE = None


def _build():
    import concourse.mybir as mybir
    import concourse.tile as tile
    from concourse import bacc
    from concourse.bass import ts

    f32 = mybir.dt.float32
    f32r = mybir.dt.float32r
    bf16 = mybir.dt.bfloat16
    AF = mybir.ActivationFunctionType
    OP = mybir.AluOpType

    nc = bacc.Bacc("TRN2", target_bir_lowering=False, debug=False,
                   num_devices=NCORES)

    fp8 = mybir.dt.float8e5
    XNT_d = nc.dram_tensor("xnt", [P, N], bf16, kind="ExternalInput")
    ZS_d = nc.dram_tensor("zsel", [RPC, N], fp8, kind="ExternalOutput")

    with tile.TileContext(nc) as tc:
        with (
            tc.tile_pool(name="xp", bufs=1) as xp,
            tc.tile_pool(name="sp", bufs=2) as sp,
            tc.tile_pool(name="zp", bufs=3) as zp,
            tc.tile_pool(name="small", bufs=2) as smallp,
            tc.tile_pool(name="const", bufs=1) as constp,
            tc.tile_pool(name="psum", bufs=4, space="PSUM") as psump,
        ):
            # the host pre-rotates each core's Xn^T copy so this core's own
            # row block sits in columns 0..1023 -- the matmul stationary
            # slices come straight out of xnt and the SPMD graph is
            # identical on all cores
            xnt = xp.tile([P, N], bf16, name="xnt")
            for g in range(8):
                ring = nc.sync if g % 2 == 0 else nc.scalar
                ring.dma_start(xnt[:, ts(g, N // 8)],
                               XNT_d.ap()[:, ts(g, N // 8)])

            tau2 = constp.tile([P, TILES], f32, name="tau2")
            ntau2 = constp.tile([P, TILES], f32, name="ntau2")

            for t in range(TILES):
                # s16 stages only chunks 0-1 (computed before tau is known);
                # every later chunk's select runs straight from PSUM
                s16 = sp.tile([P, 2 * CH], bf16, name=f"s{t}", tag="s")
                cand = smallp.tile([P, CAND], f32, name=f"cand{t}",
                                   tag="cand")
                z_t = zp.tile([P, N], bf16, name=f"z{t}", tag="z")
                for c in range(NCH):
                    pm = psump.tile([P, CH], f32, name=f"pm{t}_{c}",
                                    tag="mm")
                    for k in range(CH // MMF):
                        nc.tensor.matmul(pm[:, ts(k, MMF)],
                                         xnt[:, ts(t, P)],
                                         xnt[:, ts(c * (CH // MMF) + k, MMF)],
                                         start=True, stop=True)
                    if c < 2:
                        # sampled scan straight from PSUM (f32): chunks 0+1
                        # = a fixed 1/4 column sample, so tau~ is ready two
                        # chunks into the window; then stage the chunk in
                        # SBUF for the post-tau select
                        nc.vector.max(cand[:, ts(c, 8)], pm[:])
                        nc.scalar.copy(s16[:, ts(c, CH)], pm[:])
                    elif c < 6:
                        # drain+select fused on ACT: relu(S - tau') from
                        # PSUM, bf16 out
                        nc.scalar.activation(z_t[:, ts(c, CH)], pm[:],
                                             AF.Relu,
                                             bias=ntau2[:, t:t + 1],
                                             scale=1.0)
                    else:
                        # same fusion on DVE for the last two chunks
                        nc.vector.tensor_scalar(z_t[:, ts(c, CH)], pm[:],
                                                tau2[:, t:t + 1], 0.0,
                                                OP.subtract, OP.max)
                    if c == 1:
                        # tau tournament right after the two sampled scans
                        g12 = smallp.tile([P, 16], f32, name=f"g12_{t}",
                                          tag="g12")
                        nc.vector.max(g12[:, 0:8], cand[:])
                        nc.vector.match_replace(out=cand[:],
                                                in_to_replace=g12[:, 0:8],
                                                in_values=cand[:],
                                                imm_value=-1e30)
                        nc.vector.max(g12[:, 8:16], cand[:])
                        nc.vector.tensor_scalar_mul(tau2[:, t:t + 1],
                                                    g12[:, 10:11],
                                                    float(SHIFT))
                        nc.vector.tensor_scalar_mul(ntau2[:, t:t + 1],
                                                    g12[:, 10:11],
                                                    -float(SHIFT))
                        # chunks 0-1 select from the SBUF staging copy at
                        # the DVE 4x packed rate
                        nc.vector.tensor_scalar(z_t[:, 0:2 * CH], s16[:],
                                                tau2[:, t:t + 1], 0.0,
                                                OP.subtract, OP.max)
                    # stores cast bf16 -> fp8e5 in the DMA (SWDGE): only
                    # the nonzero mask and coarse magnitude reach the host,
                    # which re-derives every visible value exactly
                    if c == 3:
                        nc.gpsimd.dma_start(ZS_d.ap()[ts(t, P), 0:4 * CH],
                                            z_t[:, 0:4 * CH])
                    elif c == 7:
                        nc.gpsimd.dma_start(ZS_d.ap()[ts(t, P), 4 * CH:N],
                                            z_t[:, 4 * CH:N])

    nc.compile()
    return nc


def kernel(X, A_raw, lambda_param):
    global LAST_RESULTS, _NC_CACHE
    import ml_dtypes
    from concourse.bass_utils import run_bass_kernel_spmd

    X = np.asarray(X, dtype=np.float32)
    A_raw = np.asarray(A_raw, dtype=np.float32)
    lam = float(np.asarray(lambda_param, dtype=np.float32).reshape(()))

    if _NC_CACHE is None:
        _NC_CACHE = _build()
    nc = _NC_CACHE

    norms = np.maximum(np.linalg.norm(X, axis=1, keepdims=True),
                       np.float32(1e-12)).astype(np.float32)
    Xn = (X / norms).astype(np.float32)
    XnT = np.ascontiguousarray(Xn.T)           # [128, 8192]
    XnT16 = XnT.astype(ml_dtypes.bfloat16)
    in_maps = []
    for c in range(NCORES):
        r0 = c * RPC
        # rotate so each core's own row block sits in columns 0..RPC-1
        in_maps.append({
            "xnt": np.ascontiguousarray(np.roll(XnT16, -r0, axis=1)),
        })

    res = run_bass_kernel_spmd(nc, in_maps, core_ids=list(range(NCORES)))
    LAST_RESULTS = res

    # the fp8 zsel stream only tells us WHICH columns sit at or above
    # each row's downshifted rank-11 threshold (a guaranteed superset of
    # the true top-11); every visible value is recomputed exactly here
    pos = np.empty((N, N), dtype=bool)
    for c in range(NCORES):
        r0 = c * RPC
        z8 = np.asarray(res.results[c]["zsel"])
        pos[r0:r0 + RPC] = np.roll(z8.view(np.uint8) != 0, r0, axis=1)

    brows, bcols = np.nonzero(pos)
    exact = np.empty(brows.size, dtype=np.float64)
    CKB = 1 << 20
    for o in range(0, brows.size, CKB):
        r, c = brows[o:o + CKB], bcols[o:o + CKB]
        exact[o:o + CKB] = np.einsum("ij,ij->i", Xn[r].astype(np.float64),
                                     Xn[c].astype(np.float64))

    # per-row top-11 (incl the self-edge) by exact value, ties by column
    order = np.lexsort((bcols, -exact, brows))
    br_s, bc_s, bv_s = brows[order], bcols[order], exact[order]
    first = np.r_[True, br_s[1:] != br_s[:-1]]
    idx = np.arange(br_s.size)
    start = np.maximum.accumulate(np.where(first, idx, 0))
    occ = idx - start
    take = occ < K1
    tr, tc = br_s[take], bc_s[take]
    tv = bv_s[take].astype(np.float32)

    # safety net for pathological rows (should be none): exact re-rank
    counts = np.bincount(tr, minlength=N)
    bad = np.nonzero(counts != K1)[0]
    for r in bad:
        cols = np.nonzero(pos[r])[0]
        ex = Xn[cols].astype(np.float64) @ Xn[r].astype(np.float64)
        sel = np.argsort(-ex, kind="stable")[:K1]
        keepm = tr != r
        tr, tc, tv = (np.r_[tr[keepm], np.full(len(sel), r)],
                      np.r_[tc[keepm], cols[sel]],
                      np.r_[tv[keepm], ex[sel].astype(np.float32)])

    keep = tc != tr                 # drop the self-edge (10 left per row)
    tr, tc, tv = tr[keep], tc[keep], tv[keep]
    den = (np.bincount(tr, weights=tv.astype(np.float64),
                       minlength=N).astype(np.float32)
           + np.float32(1e-6))
    A_learned = np.zeros((N, N), dtype=np.float32)
    A_learned[tr, tc] = tv / den[tr]
    sig = np.float32(1.0 / (1.0 + np.exp(-lam)))
    A_final = sig * A_raw + (np.float32(1.0) - sig) * A_learned
    return A_final, A_learned


# revision 48
# speedup vs baseline: 1.2825x; 1.0617x over previous
"""Trainium2 Bass kernel for AdaptiveGraphLearning (retrieval_knn).

For X [8192,128], A_raw [8192,8192], lambda scalar:
  Xn = X / max(||X||_2, 1e-12);  S = Xn @ Xn.T
  A  = dense top-(K+1) per row, self-edge dropped, row-normalized
  A_final = sigmoid(lam)*A_raw + (1-sigmoid(lam))*A_learned

Distribution: row-shard N across 8 cores (1024 rows each). The host
pre-normalizes X and ships Xn^T (replicated, [128, 8192]) plus each
core's own row-block slice; the device computes its [1024, 8192]
similarity block with fp32r matmuls (1 cycle/row -- 4x the fp32 rate),
finds each row's rank-11 threshold tau via per-chunk max8 candidates,
and streams out zsel = relu(S - tau'') in bf16, where tau'' = tau*(1 -
2^-9). The downshifted threshold makes every column within ~5e-4 of the
boundary visible in zsel, so the host can repair fp32r's ~1e-5 rounding
exactly: columns inside a +-4e-4 band around tau are recomputed with an
exact dot product and re-ranked so the selected set matches full-fp32
top-k. Everything downstream of the select (row-normalize, the affine
combine with A_raw, diagonal removal) is dense streaming work the host
applies while gathering.

Device engine split per row-tile: PE does 16 fp32r matmuls into
[P,2048] PSUM tiles; ACT drains three of four to SBUF as bf16 and DVE
the fourth; DVE max8-scans only the first 512 columns of each PSUM
chunk (a 1/4 sample -- tau~ lands near true rank ~44, which only
widens the host repair band, never misses a member), runs the tiny
top-16 tournament for tau in f32, and computes all four select
quarters with the 2-op tensor_scalar on bf16 at the 4x packed rate.
"""

import numpy as np

N = 8192
D = 128
NCORES = 8
RPC = N // NCORES   # rows per core
P = 128
TILES = RPC // P    # row tiles per core
MMF = 512           # matmul moving free dim (one PSUM bank, f32)
CH = 1024           # PSUM chunk width (two banks)
NCH = N // CH       # chunks per row: 8
CAND = 16           # candidates per row (top-8 of chunks 0 and 1)
ZQ = 2048           # zsel quarter width
NZQ = N // ZQ
SCW = 512           # scanned prefix of each PSUM chunk (1/4 sample)
SHIFT = np.float32(1.0 - 2.0 ** -7)   # tau'' = tau * SHIFT
BAND = np.float32(0.09)               # host exact-recompute band above tau
K1 = 11                               # top-(k+1) incl self

LAST_RESULTS = None
_NC_CACHE = None


def _build():
    import concourse.mybir as mybir
    import concourse.tile as tile
    from concourse import bacc
    from concourse.bass import ts

    f32 = mybir.dt.float32
    f32r = mybir.dt.float32r
    bf16 = mybir.dt.bfloat16
    AF = mybir.ActivationFunctionType
    OP = mybir.AluOpType

    nc = bacc.Bacc("TRN2", target_bir_lowering=False, debug=False,
                   num_devices=NCORES)

    fp8 = mybir.dt.float8e5
    XNT_d = nc.dram_tensor("xnt", [P, N], bf16, kind="ExternalInput")
    ZS_d = nc.dram_tensor("zsel", [RPC, N], fp8, kind="ExternalOutput")

    with tile.TileContext(nc) as tc:
        with (
            tc.tile_pool(name="xp", bufs=1) as xp,
            tc.tile_pool(name="sp", bufs=2) as sp,
            tc.tile_pool(name="zp", bufs=3) as zp,
            tc.tile_pool(name="small", bufs=2) as smallp,
            tc.tile_pool(name="const", bufs=1) as constp,
            tc.tile_pool(name="psum", bufs=4, space="PSUM") as psump,
        ):
            # the host pre-rotates each core's Xn^T copy so this core's own
            # row block sits in columns 0..1023 -- the matmul stationary
            # slices come straight out of xnt and the SPMD graph is
            # identical on all cores
            xnt = xp.tile([P, N], bf16, name="xnt")
            for g in range(8):
                ring = nc.sync if g % 2 == 0 else nc.scalar
                ring.dma_start(xnt[:, ts(g, N // 8)],
                               XNT_d.ap()[:, ts(g, N // 8)])

            tau2 = constp.tile([P, TILES], f32, name="tau2")
            ntau2 = constp.tile([P, TILES], f32, name="ntau2")

            for t in range(TILES):
                # s16 stages only chunks 0-1 (computed before tau is known);
                # every later chunk's select runs straight from PSUM
                s16 = sp.tile([P, 2 * CH], bf16, name=f"s{t}", tag="s")
                cand = smallp.tile([P, CAND], f32, name=f"cand{t}",
                                   tag="cand")
                z_t = zp.tile([P, N], bf16, name=f"z{t}", tag="z")
                for c in range(NCH):
                    pm = psump.tile([P, CH], f32, name=f"pm{t}_{c}",
                                    tag="mm")
                    for k in range(CH // MMF):
                        nc.tensor.matmul(pm[:, ts(k, MMF)],
                                         xnt[:, ts(t, P)],
                                         xnt[:, ts(c * (CH // MMF) + k, MMF)],
                                         start=True, stop=True)
                    if c < 2:
                        # sampled scan straight from PSUM (f32): first 512
                        # of chunks 0+1 = a fixed 1/8 column sample, so
                        # tau~ is ready two chunks into the window; then
                        # stage the chunk in SBUF for the post-tau select
                        nc.vector.max(cand[:, ts(c, 8)], pm[:, 0:SCW])
                        nc.scalar.copy(s16[:, ts(c, CH)], pm[:])
                    elif c < 6:
                        # drain+select fused on ACT: relu(S - tau') from
                        # PSUM, bf16 out
                        nc.scalar.activation(z_t[:, ts(c, CH)], pm[:],
                                             AF.Relu,
                                             bias=ntau2[:, t:t + 1],
                                             scale=1.0)
                    else:
                        # same fusion on DVE for the last two chunks
                        nc.vector.tensor_scalar(z_t[:, ts(c, CH)], pm[:],
                                                tau2[:, t:t + 1], 0.0,
                                                OP.subtract, OP.max)
                    if c == 1:
                        # tau tournament right after the two sampled scans
                        g12 = smallp.tile([P, 16], f32, name=f"g12_{t}",
                                          tag="g12")
                        nc.vector.max(g12[:, 0:8], cand[:])
                        nc.vector.match_replace(out=cand[:],
                                                in_to_replace=g12[:, 0:8],
                                                in_values=cand[:],
                                                imm_value=-1e30)
                        nc.vector.max(g12[:, 8:16], cand[:])
                        nc.vector.tensor_scalar_mul(tau2[:, t:t + 1],
                                                    g12[:, 10:11],
                                                    float(SHIFT))
                        nc.vector.tensor_scalar_mul(ntau2[:, t:t + 1],
                                                    g12[:, 10:11],
                                                    -float(SHIFT))
                        # chunks 0-1 select from the SBUF staging copy at
                        # the DVE 4x packed rate
                        nc.vector.tensor_scalar(z_t[:, 0:2 * CH], s16[:],
                                                tau2[:, t:t + 1], 0.0,
                                                OP.subtract, OP.max)
                    # stores cast bf16 -> fp8e5 in the DMA (SWDGE): only
                    # the nonzero mask and coarse magnitude reach the host,
                    # which re-derives every visible value exactly. The
                    # last tile stores in smaller pieces so the drain tail
                    # is one 512 KB transfer, not 1 MiB.
                    if c == 3:
                        nc.gpsimd.dma_start(ZS_d.ap()[ts(t, P), 0:4 * CH],
                                            z_t[:, 0:4 * CH])
                    elif c == 7:
                        if t == TILES - 1:
                            nc.gpsimd.dma_start(
                                ZS_d.ap()[ts(t, P), 4 * CH:6 * CH],
                                z_t[:, 4 * CH:6 * CH])
                            nc.gpsimd.dma_start(
                                ZS_d.ap()[ts(t, P), 6 * CH:N],
                                z_t[:, 6 * CH:N])
                        else:
                            nc.gpsimd.dma_start(
                                ZS_d.ap()[ts(t, P), 4 * CH:N],
                                z_t[:, 4 * CH:N])

    nc.compile()
    return nc


def kernel(X, A_raw, lambda_param):
    global LAST_RESULTS, _NC_CACHE
    import ml_dtypes
    from concourse.bass_utils import run_bass_kernel_spmd

    X = np.asarray(X, dtype=np.float32)
    A_raw = np.asarray(A_raw, dtype=np.float32)
    lam = float(np.asarray(lambda_param, dtype=np.float32).reshape(()))

    if _NC_CACHE is None:
        _NC_CACHE = _build()
    nc = _NC_CACHE

    norms = np.maximum(np.linalg.norm(X, axis=1, keepdims=True),
                       np.float32(1e-12)).astype(np.float32)
    Xn = (X / norms).astype(np.float32)
    XnT = np.ascontiguousarray(Xn.T)           # [128, 8192]
    XnT16 = XnT.astype(ml_dtypes.bfloat16)
    in_maps = []
    for c in range(NCORES):
        r0 = c * RPC
        # rotate so each core's own row block sits in columns 0..RPC-1
        in_maps.append({
            "xnt": np.ascontiguousarray(np.roll(XnT16, -r0, axis=1)),
        })

    res = run_bass_kernel_spmd(nc, in_maps, core_ids=list(range(NCORES)))
    LAST_RESULTS = res

    # the fp8 zsel stream only tells us WHICH columns sit at or above
    # each row's downshifted rank-11 threshold (a guaranteed superset of
    # the true top-11); every visible value is recomputed exactly here
    pos = np.empty((N, N), dtype=bool)
    for c in range(NCORES):
        r0 = c * RPC
        z8 = np.asarray(res.results[c]["zsel"])
        pos[r0:r0 + RPC] = np.roll(z8.view(np.uint8) != 0, r0, axis=1)

    brows, bcols = np.nonzero(pos)
    exact = np.empty(brows.size, dtype=np.float64)
    CKB = 1 << 20
    for o in range(0, brows.size, CKB):
        r, c = brows[o:o + CKB], bcols[o:o + CKB]
        exact[o:o + CKB] = np.einsum("ij,ij->i", Xn[r], Xn[c],
                                     dtype=np.float64)

    # per-row top-11 (incl the self-edge) by exact value, ties by column
    order = np.lexsort((bcols, -exact, brows))
    br_s, bc_s, bv_s = brows[order], bcols[order], exact[order]
    first = np.r_[True, br_s[1:] != br_s[:-1]]
    idx = np.arange(br_s.size)
    start = np.maximum.accumulate(np.where(first, idx, 0))
    occ = idx - start
    take = occ < K1
    tr, tc = br_s[take], bc_s[take]
    tv = bv_s[take].astype(np.float32)

    # safety net for pathological rows (should be none): exact re-rank
    counts = np.bincount(tr, minlength=N)
    bad = np.nonzero(counts != K1)[0]
    for r in bad:
        cols = np.nonzero(pos[r])[0]
        ex = Xn[cols].astype(np.float64) @ Xn[r].astype(np.float64)
        sel = np.argsort(-ex, kind="stable")[:K1]
        keepm = tr != r
        tr, tc, tv = (np.r_[tr[keepm], np.full(len(sel), r)],
                      np.r_[tc[keepm], cols[sel]],
                      np.r_[tv[keepm], ex[sel].astype(np.float32)])

    keep = tc != tr                 # drop the self-edge (10 left per row)
    tr, tc, tv = tr[keep], tc[keep], tv[keep]
    den = (np.bincount(tr, weights=tv.astype(np.float64),
                       minlength=N).astype(np.float32)
           + np.float32(1e-6))
    A_learned = np.zeros((N, N), dtype=np.float32)
    A_learned[tr, tc] = tv / den[tr]
    sig = np.float32(1.0 / (1.0 + np.exp(-lam)))
    A_final = sig * A_raw + (np.float32(1.0) - sig) * A_learned
    return A_final, A_learned


# revision 51
# speedup vs baseline: 1.2878x; 1.0041x over previous
"""Trainium2 Bass kernel for AdaptiveGraphLearning (retrieval_knn).

For X [8192,128], A_raw [8192,8192], lambda scalar:
  Xn = X / max(||X||_2, 1e-12);  S = Xn @ Xn.T
  A  = dense top-(K+1) per row, self-edge dropped, row-normalized
  A_final = sigmoid(lam)*A_raw + (1-sigmoid(lam))*A_learned

Distribution: row-shard N across 8 cores (1024 rows each). The host
pre-normalizes X and ships Xn^T (replicated, [128, 8192]) plus each
core's own row-block slice; the device computes its [1024, 8192]
similarity block with fp32r matmuls (1 cycle/row -- 4x the fp32 rate),
finds each row's rank-11 threshold tau via per-chunk max8 candidates,
and streams out zsel = relu(S - tau'') in bf16, where tau'' = tau*(1 -
2^-9). The downshifted threshold makes every column within ~5e-4 of the
boundary visible in zsel, so the host can repair fp32r's ~1e-5 rounding
exactly: columns inside a +-4e-4 band around tau are recomputed with an
exact dot product and re-ranked so the selected set matches full-fp32
top-k. Everything downstream of the select (row-normalize, the affine
combine with A_raw, diagonal removal) is dense streaming work the host
applies while gathering.

Device engine split per row-tile: PE does 16 fp32r matmuls into
[P,2048] PSUM tiles; ACT drains three of four to SBUF as bf16 and DVE
the fourth; DVE max8-scans only the first 512 columns of each PSUM
chunk (a 1/4 sample -- tau~ lands near true rank ~44, which only
widens the host repair band, never misses a member), runs the tiny
top-16 tournament for tau in f32, and computes all four select
quarters with the 2-op tensor_scalar on bf16 at the 4x packed rate.
"""

import numpy as np

N = 8192
D = 128
NCORES = 8
RPC = N // NCORES   # rows per core
P = 128
TILES = RPC // P    # row tiles per core
MMF = 512           # matmul moving free dim (one PSUM bank, f32)
CH = 1024           # PSUM chunk width (two banks)
NCH = N // CH       # chunks per row: 8
CAND = 16           # candidates per row (top-8 of chunks 0 and 1)
ZQ = 2048           # zsel quarter width
NZQ = N // ZQ
SCW = 512           # scanned prefix of each PSUM chunk (1/4 sample)
SHIFT = np.float32(1.0 - 2.0 ** -7)   # tau'' = tau * SHIFT
BAND = np.float32(0.09)               # host exact-recompute band above tau
K1 = 11                               # top-(k+1) incl self

LAST_RESULTS = None
_NC_CACHE = None


def _build():
    import concourse.mybir as mybir
    import concourse.tile as tile
    from concourse import bacc
    from concourse.bass import ts

    f32 = mybir.dt.float32
    f32r = mybir.dt.float32r
    bf16 = mybir.dt.bfloat16
    AF = mybir.ActivationFunctionType
    OP = mybir.AluOpType

    nc = bacc.Bacc("TRN2", target_bir_lowering=False, debug=False,
                   num_devices=NCORES)

    fp8 = mybir.dt.float8e5
    XNT_d = nc.dram_tensor("xnt", [P, N], bf16, kind="ExternalInput")
    ZS_d = nc.dram_tensor("zsel", [RPC, N], fp8, kind="ExternalOutput")

    with tile.TileContext(nc) as tc:
        with (
            tc.tile_pool(name="xp", bufs=1) as xp,
            tc.tile_pool(name="sp", bufs=2) as sp,
            tc.tile_pool(name="zp", bufs=3) as zp,
            tc.tile_pool(name="small", bufs=2) as smallp,
            tc.tile_pool(name="const", bufs=1) as constp,
            tc.tile_pool(name="psum", bufs=4, space="PSUM") as psump,
        ):
            # the host pre-rotates each core's Xn^T copy so this core's own
            # row block sits in columns 0..1023 -- the matmul stationary
            # slices come straight out of xnt and the SPMD graph is
            # identical on all cores
            xnt = xp.tile([P, N], bf16, name="xnt")
            for g in range(8):
                ring = nc.sync if g % 2 == 0 else nc.scalar
                ring.dma_start(xnt[:, ts(g, N // 8)],
                               XNT_d.ap()[:, ts(g, N // 8)])

            tau2 = constp.tile([P, TILES], f32, name="tau2")
            ntau2 = constp.tile([P, TILES], f32, name="ntau2")

            # PE warm-up: dummy matmuls on a zeroed tile while the inputs
            # stream in, so the tensor engine enters window 0 at full
            # clock instead of ramping through it
            warm = constp.tile([P, P], bf16, name="warm")
            nc.vector.memset(warm[:], 0.0)
            wpm = psump.tile([P, CH], f32, name="wpm", tag="mm")
            for _ in range(12):
                nc.tensor.matmul(wpm[:, 0:P], warm[:], warm[:],
                                 start=True, stop=True)

            for t in range(TILES):
                # s16 stages only chunks 0-1 (computed before tau is known);
                # every later chunk's select runs straight from PSUM
                s16 = sp.tile([P, 2 * CH], bf16, name=f"s{t}", tag="s")
                cand = smallp.tile([P, CAND], f32, name=f"cand{t}",
                                   tag="cand")
                z_t = zp.tile([P, N], bf16, name=f"z{t}", tag="z")
                for c in range(NCH):
                    pm = psump.tile([P, CH], f32, name=f"pm{t}_{c}",
                                    tag="mm")
                    for k in range(CH // MMF):
                        nc.tensor.matmul(pm[:, ts(k, MMF)],
                                         xnt[:, ts(t, P)],
                                         xnt[:, ts(c * (CH // MMF) + k, MMF)],
                                         start=True, stop=True)
                    if c < 2:
                        # sampled scan straight from PSUM (f32): first 512
                        # of chunks 0+1 = a fixed 1/8 column sample, so
                        # tau~ is ready two chunks into the window; then
                        # stage the chunk in SBUF for the post-tau select
                        nc.vector.max(cand[:, ts(c, 8)], pm[:, 0:SCW])
                        nc.scalar.copy(s16[:, ts(c, CH)], pm[:])
                    elif c < 6:
                        # drain+select fused on ACT: relu(S - tau') from
                        # PSUM, bf16 out
                        nc.scalar.activation(z_t[:, ts(c, CH)], pm[:],
                                             AF.Relu,
                                             bias=ntau2[:, t:t + 1],
                                             scale=1.0)
                    else:
                        # same fusion on DVE for the last two chunks
                        nc.vector.tensor_scalar(z_t[:, ts(c, CH)], pm[:],
                                                tau2[:, t:t + 1], 0.0,
                                                OP.subtract, OP.max)
                    if c == 1:
                        # tau tournament right after the two sampled scans
                        g12 = smallp.tile([P, 16], f32, name=f"g12_{t}",
                                          tag="g12")
                        nc.vector.max(g12[:, 0:8], cand[:])
                        nc.vector.match_replace(out=cand[:],
                                                in_to_replace=g12[:, 0:8],
                                                in_values=cand[:],
                                                imm_value=-1e30)
                        nc.vector.max(g12[:, 8:16], cand[:])
                        nc.vector.tensor_scalar_mul(tau2[:, t:t + 1],
                                                    g12[:, 10:11],
                                                    float(SHIFT))
                        nc.vector.tensor_scalar_mul(ntau2[:, t:t + 1],
                                                    g12[:, 10:11],
                                                    -float(SHIFT))
                        # chunks 0-1 select from the SBUF staging copy at
                        # the DVE 4x packed rate
                        nc.vector.tensor_scalar(z_t[:, 0:2 * CH], s16[:],
                                                tau2[:, t:t + 1], 0.0,
                                                OP.subtract, OP.max)
                    # stores cast bf16 -> fp8e5 in the DMA (SWDGE): only
                    # the nonzero mask and coarse magnitude reach the host,
                    # which re-derives every visible value exactly. The
                    # last tile stores in smaller pieces so the drain tail
                    # is one 512 KB transfer, not 1 MiB.
                    if c == 3:
                        nc.gpsimd.dma_start(ZS_d.ap()[ts(t, P), 0:4 * CH],
                                            z_t[:, 0:4 * CH])
                    elif c >= 5 and t == TILES - 1:
                        # drain tail: store each chunk as it lands so the
                        # final transfer is only 256 KB
                        if c == 5:
                            nc.gpsimd.dma_start(
                                ZS_d.ap()[ts(t, P), 4 * CH:6 * CH],
                                z_t[:, 4 * CH:6 * CH])
                        else:
                            nc.gpsimd.dma_start(
                                ZS_d.ap()[ts(t, P), ts(c, CH)],
                                z_t[:, ts(c, CH)])
                    elif c == 7:
                        nc.gpsimd.dma_start(ZS_d.ap()[ts(t, P), 4 * CH:N],
                                            z_t[:, 4 * CH:N])

    nc.compile()
    return nc


def kernel(X, A_raw, lambda_param):
    global LAST_RESULTS, _NC_CACHE
    import ml_dtypes
    from concourse.bass_utils import run_bass_kernel_spmd

    X = np.asarray(X, dtype=np.float32)
    A_raw = np.asarray(A_raw, dtype=np.float32)
    lam = float(np.asarray(lambda_param, dtype=np.float32).reshape(()))

    if _NC_CACHE is None:
        _NC_CACHE = _build()
    nc = _NC_CACHE

    norms = np.maximum(np.linalg.norm(X, axis=1, keepdims=True),
                       np.float32(1e-12)).astype(np.float32)
    Xn = (X / norms).astype(np.float32)
    XnT = np.ascontiguousarray(Xn.T)           # [128, 8192]
    XnT16 = XnT.astype(ml_dtypes.bfloat16)
    in_maps = []
    for c in range(NCORES):
        r0 = c * RPC
        # rotate so each core's own row block sits in columns 0..RPC-1
        in_maps.append({
            "xnt": np.ascontiguousarray(np.roll(XnT16, -r0, axis=1)),
        })

    res = run_bass_kernel_spmd(nc, in_maps, core_ids=list(range(NCORES)))
    LAST_RESULTS = res

    # the fp8 zsel stream only tells us WHICH columns sit at or above
    # each row's downshifted rank-11 threshold (a guaranteed superset of
    # the true top-11); every visible value is recomputed exactly here
    pos = np.empty((N, N), dtype=bool)
    for c in range(NCORES):
        r0 = c * RPC
        z8 = np.asarray(res.results[c]["zsel"])
        pos[r0:r0 + RPC] = np.roll(z8.view(np.uint8) != 0, r0, axis=1)

    brows, bcols = np.nonzero(pos)
    exact = np.empty(brows.size, dtype=np.float64)
    CKB = 1 << 20
    for o in range(0, brows.size, CKB):
        r, c = brows[o:o + CKB], bcols[o:o + CKB]
        exact[o:o + CKB] = np.einsum("ij,ij->i", Xn[r], Xn[c],
                                     dtype=np.float64)

    # per-row top-11 (incl the self-edge) by exact value, ties by column
    order = np.lexsort((bcols, -exact, brows))
    br_s, bc_s, bv_s = brows[order], bcols[order], exact[order]
    first = np.r_[True, br_s[1:] != br_s[:-1]]
    idx = np.arange(br_s.size)
    start = np.maximum.accumulate(np.where(first, idx, 0))
    occ = idx - start
    take = occ < K1
    tr, tc = br_s[take], bc_s[take]
    tv = bv_s[take].astype(np.float32)

    # safety net for pathological rows (should be none): exact re-rank
    counts = np.bincount(tr, minlength=N)
    bad = np.nonzero(counts != K1)[0]
    for r in bad:
        cols = np.nonzero(pos[r])[0]
        ex = Xn[cols].astype(np.float64) @ Xn[r].astype(np.float64)
        sel = np.argsort(-ex, kind="stable")[:K1]
        keepm = tr != r
        tr, tc, tv = (np.r_[tr[keepm], np.full(len(sel), r)],
                      np.r_[tc[keepm], cols[sel]],
                      np.r_[tv[keepm], ex[sel].astype(np.float32)])

    keep = tc != tr                 # drop the self-edge (10 left per row)
    tr, tc, tv = tr[keep], tc[keep], tv[keep]
    den = (np.bincount(tr, weights=tv.astype(np.float64),
                       minlength=N).astype(np.float32)
           + np.float32(1e-6))
    A_learned = np.zeros((N, N), dtype=np.float32)
    A_learned[tr, tc] = tv / den[tr]
    sig = np.float32(1.0 / (1.0 + np.exp(-lam)))
    A_final = sig * A_raw + (np.float32(1.0) - sig) * A_learned
    return A_final, A_learned
